# revision 37
# baseline (speedup 1.0000x reference)
"""Trainium2 Bass kernel for 16-head MHA (B=4, S=2048, E=1024, fp32 io).

Sharding: 8 cores = (batch b, head-half hh) grid. Core c handles batch
c // 2 and heads [hh*8, hh*8+8) (d-slice of 512 channels). Each core
computes a partial y_c = attn_out_slice @ Wo_slice.T of the full (S, E)
output; the host sums core pairs and adds bo.

v2 layout choices (bf16 data, fp32 PSUM accumulation):
  - All inputs (xT, weights) are converted to bf16 host-side: halves
    HBM traffic, SBUF footprint and DVE evac cost; PE matmul throughput
    is dtype-independent at these tile shapes (1 cycle/row).
  - V projection runs e-outer in two 8-bank PSUM passes so matmuls
    start as soon as the first xT e-chunk lands instead of waiting for
    the full xT DMA.
  - Scores (K=64 per head) are emitted as a row-tiled pair: head0 on
    PE rows 0-63, head1 on rows 64-127 (tile_position auto-derived
    from base partitions) -> the two matmuls run concurrently.
  - k-loop is software-pipelined with a one-iteration skew: PV for
    chunk k-1 is emitted after scores+exp of chunk k, so the PE never
    waits on the current exp (ACT) and both engines stream.
  - Softmax denominator rides the PV matmul as a 65th output row
    (ones column appended to V); normalization via ones-matmul
    replicate + reciprocal + multiply on DVE.
  - Q/K projections and the output projection are spread as fill work
    across the k-loop slots, as in v1.
"""
import numpy as np
import ml_dtypes

import concourse.bass as bass
import concourse.mybir as mybir
import concourse.tile as tile
from concourse import bacc
from concourse.bass_utils import run_bass_kernel_spmd

B, S, E = 4, 2048, 1024
DLOC = 512          # head-dim channels per core (8 heads)
NJ = DLOC // 128    # 4 j-chunks (head pairs)
NE = E // 128       # 8 e-chunks
NSC = S // 128      # 16 s-chunks
NQC = S // 512      # 4 q-chunks
NKC = S // 128      # 16 k-chunks
F32 = mybir.dt.float32
F32R = mybir.dt.float32r
BF16 = mybir.dt.bfloat16
I16 = mybir.dt.int16
EXP = mybir.ActivationFunctionType.Exp
NPBF16 = ml_dtypes.bfloat16
# Schraudolph fast-exp constants emitting bf16 bits via int16:
# bf16_bits(e^(s/8)) ~= int16(s * log2(e)*128/8 + (127 - 0.043677)*128)
SCH_A = float(np.log2(np.e) * 128.0 / 8.0)
SCH_B = float((127.0 - 0.043677) * 128.0 + 0.5)

_CACHED = {}


def _build(loop_k=None):
    nc = bacc.Bacc()
    xT = nc.declare_dram_parameter("xT", [E, S], BF16, isOutput=False)
    wqT = nc.declare_dram_parameter("wqT", [E, DLOC], BF16, isOutput=False)
    wkT = nc.declare_dram_parameter("wkT", [E, DLOC], BF16, isOutput=False)
    wvT = nc.declare_dram_parameter("wvT", [E, DLOC], BF16, isOutput=False)
    woT = nc.declare_dram_parameter("woT", [DLOC, E], BF16, isOutput=False)
    bq = nc.declare_dram_parameter("bq", [DLOC, 1], F32, isOutput=False)
    bk = nc.declare_dram_parameter("bk", [DLOC, 1], F32, isOutput=False)
    bv = nc.declare_dram_parameter("bv", [DLOC, 1], F32, isOutput=False)
    ones = nc.declare_dram_parameter("ones", [128, 64], F32R, isOutput=False)
    y = nc.declare_dram_parameter("y", [S, E], F32, isOutput=True)

    with tile.TileContext(nc) as tc:
        with (
            tc.tile_pool(name="big", bufs=1) as big,
            tc.tile_pool(name="wpool", bufs=1) as wpool,
            tc.tile_pool(name="cons", bufs=1) as cons,
            tc.tile_pool(name="qpool", bufs=1) as qpool,
            tc.tile_pool(name="opool", bufs=2) as opool,
            tc.tile_pool(name="ppool", bufs=6) as ppool,
            tc.tile_pool(name="dpool", bufs=1) as dpool,
            tc.tile_pool(name="ypool", bufs=1) as ypool,
            tc.tile_pool(name="ps_proj", bufs=2, space="PSUM") as ps_proj,
            tc.tile_pool(name="ps_sc", bufs=2, space="PSUM") as ps_sc,
            tc.tile_pool(name="ps_pv", bufs=2, space="PSUM") as ps_pv,
        ):
            # ---- constants (single strided DMA per tensor) ----
            ones64 = cons.tile([128, 64], F32R)
            bq_t = cons.tile([128, NJ], F32)
            bk_t = cons.tile([128, NJ], F32)
            bv_t = cons.tile([128, NJ], F32)
            bvh1_t = cons.tile([64, NJ], F32)

            def load_consts():
                nc.sync.dma_start(out=ones64, in_=ones[:, :])
                nc.sync.dma_start(
                    out=bq_t, in_=bq.rearrange("(j p) o -> p (j o)", p=128))
                nc.sync.dma_start(
                    out=bk_t, in_=bk.rearrange("(j p) o -> p (j o)", p=128))
                nc.sync.dma_start(
                    out=bv_t, in_=bv.rearrange("(j p) o -> p (j o)", p=128))
                nc.sync.dma_start(
                    out=bvh1_t,
                    in_=bv.rearrange("(j h p) o -> h p (j o)", j=NJ, h=2)[1])

            # ---- optional on-device repeat loop (timing only) ----
            import contextlib
            loop_cm = tc.For_i(0, loop_k) if loop_k else contextlib.nullcontext()
            with loop_cm:
                _body(nc, tc, locals())

    nc.compile()
    return nc


def _body(nc, tc, env):
    xT, wqT, wkT, wvT, woT = env["xT"], env["wqT"], env["wkT"], env["wvT"], env["woT"]
    y = env["y"]
    big, wpool, qpool, opool = env["big"], env["wpool"], env["qpool"], env["opool"]
    ppool, dpool, ypool = env["ppool"], env["dpool"], env["ypool"]
    ps_proj, ps_sc, ps_pv = env["ps_proj"], env["ps_sc"], env["ps_pv"]
    ones64 = env["ones64"]
    bq_t, bk_t, bv_t, bvh1_t = (env["bq_t"], env["bk_t"], env["bv_t"],
                                env["bvh1_t"])

    # ---- load wv, then xT chunk-by-chunk (V proj overlaps the DMA) ----
    wv_t = wpool.tile([128, NE, DLOC], BF16, tag="wv")
    nc.sync.dma_start(out=wv_t, in_=wvT.rearrange("(e p) d -> p e d", p=128))
    xt_list = []
    for e in range(NE):
        xe = big.tile([128, S], BF16, tag=f"xt{e}", bufs=2)
        nc.sync.dma_start(out=xe, in_=xT[e*128:(e+1)*128, :])
        xt_list.append(xe)
    wk_t = wpool.tile([128, NE, DLOC], BF16, tag="wk")
    nc.sync.dma_start(out=wk_t, in_=wkT.rearrange("(e p) d -> p e d", p=128))
    wq_t = wpool.tile([128, NE, DLOC], BF16, tag="wq")
    nc.sync.dma_start(out=wq_t, in_=wqT.rearrange("(e p) d -> p e d", p=128))
    wo_t = wpool.tile([128, NJ, E], BF16, tag="wo")
    nc.sync.dma_start(out=wo_t, in_=woT.rearrange("(j p) d -> p j d", p=128))
    env["load_consts"]()  # queued behind wv/xT so they don't delay V-proj

    # ---- V projection, e-outer in two 8-bank passes ----
    # vt[:, sc, j, h, 0:64] = V columns; vt[:, sc, j, h, 64] = 1.0 so the
    # PV matmul's 65th output row accumulates the softmax denominator.
    vt = big.tile([128, NSC, NJ, 2, 65], BF16)
    nc.vector.memset(vt[:, :, :, :, 64:65], 1.0)
    # The k-loop PSUM pools are idle during the prologue, so borrow their
    # banks for eight concurrent e-outer accumulation groups per pass.
    for half in range(2):
        t1 = ps_sc.tile([128, 2, 512], F32, tag="sc")
        t2 = ps_sc.tile([128, 2, 512], F32, tag="sc")
        t3 = ps_pv.tile([128, 512], F32, tag="pv")
        t4 = ps_pv.tile([128, 512], F32, tag="pv")
        t5 = ps_proj.tile([128, 512], F32, tag="proj")
        t6 = ps_proj.tile([128, 512], F32, tag="proj")
        groups = [t1[:, 0, :], t1[:, 1, :], t2[:, 0, :], t2[:, 1, :],
                  t3, t4, t5, t6]
        for e in range(NE):
            for scl in range(8):
                sc = half * 8 + scl
                nc.tensor.matmul(
                    groups[scl], xt_list[e][:, sc*128:(sc+1)*128],
                    wv_t[:, e, :], start=(e == 0), stop=(e == NE - 1))
        for scl in range(8):
            sc = half * 8 + scl
            nc.vector.tensor_copy(
                vt[:, sc, :, :, 0:64],
                groups[scl].rearrange("p (j h c) -> p j h c", j=NJ, h=2))

    oct_ = big.tile([128, NJ, S], BF16)

    # ---- main loop: j (head pairs) outer, q-chunks inner ----
    # Projections are software-pipelined into the attention k-loop ("fill"
    # slots) so the statically-scheduled PE stream never starves ACT.
    def k_proj_group(j, qc, w_t, b_t, dest_fn, ps_fn=None):
        cell = {}
        def get_pk():
            if "pk" not in cell:
                if ps_fn is not None:
                    cell["pk"] = ps_fn()
                else:
                    pk_lazy = ps_proj.tile([128, 512], F32, tag="proj")
                    cell["pk"] = pk_lazy
            return cell["pk"]
        mms = [lambda e=e: nc.tensor.matmul(
                   get_pk(), w_t[:, e, j*128:(j+1)*128],
                   xt_list[e][:, qc*512:(qc+1)*512],
                   start=(e == 0), stop=(e == NE - 1)) for e in range(NE)]
        def evac():
            nc.vector.tensor_scalar_add(dest_fn(), get_pk(), b_t[:, j:j+1])
        return mms, evac

    def o_proj_sc(sc):
        cell = {}
        def get_ysb():
            if "ysb" not in cell:
                ysb_lazy = ypool.tile([128, E], F32, tag="y", bufs=2)
                cell["ysb"] = ysb_lazy
            return cell["ysb"]
        def get_py(eh):
            key = f"py{eh}"
            if key not in cell:
                py_lazy = ps_proj.tile([128, 512], F32, tag="proj")
                cell[key] = py_lazy
            return cell[key]
        steps = []
        for eh in range(2):
            for jj in range(NJ):
                steps.append(lambda jj=jj, eh=eh: nc.tensor.matmul(
                    get_py(eh), oct_[:, jj, sc*128:(sc+1)*128],
                    wo_t[:, jj, eh*512:(eh+1)*512],
                    start=(jj == 0), stop=(jj == NJ - 1)))
            steps.append(lambda eh=eh: nc.vector.tensor_copy(
                get_ysb()[:, eh*512:(eh+1)*512], get_py(eh)))
        steps.append(lambda: nc.sync.dma_start(
            out=y[sc*128:(sc+1)*128, :], in_=get_ysb()))
        return steps

    # K-projection for j=0 and Q-projection for (0, 0) run up front.
    # Borrow the idle scores-psum slots so all five groups pipeline
    # without waiting on the two ps_proj slots.
    ka = ps_sc.tile([128, 2, 512], F32, tag="sc")
    kb = ps_sc.tile([128, 2, 512], F32, tag="sc")
    kt_next = qpool.tile([128, S], BF16, tag="kt", bufs=2)
    for qc in range(NQC):
        mms, evac = k_proj_group(
            0, qc, wk_t, bk_t,
            (lambda qc=qc, t=kt_next: t[:, qc*512:(qc+1)*512]),
            ps_fn=(lambda qc=qc: (ka, kb)[qc // 2][:, qc % 2, :]))
        for m in mms:
            m()
        evac()
    qt_next = qpool.tile([128, 512], BF16, tag="qt", bufs=2)
    mms, evac = k_proj_group(
        0, 0, wq_t, bq_t, (lambda t=qt_next: t[:, :]))
    for m in mms:
        m()
    evac()

    for j in range(NJ):
        kt = kt_next
        if j < NJ - 1:
            kt_next = qpool.tile([128, S], BF16, tag="kt", bufs=2)
        for qc in range(NQC):
            qt = qt_next
            # fill work emitted one step per k iteration
            fills = []
            if qc < NQC - 1:
                qt_next = qpool.tile([128, 512], BF16, tag="qt", bufs=2)
                mms, evac = k_proj_group(
                    j, qc + 1, wq_t, bq_t, (lambda t=qt_next: t[:, :]))
                fills.extend(mms); fills.append(evac)
            elif j < NJ - 1:
                qt_next = qpool.tile([128, 512], BF16, tag="qt", bufs=2)
                mms, evac = k_proj_group(
                    j + 1, 0, wq_t, bq_t, (lambda t=qt_next: t[:, :]))
                fills.extend(mms); fills.append(evac)
            if j < NJ - 1:
                mms, evac = k_proj_group(
                    j + 1, qc, wk_t, bk_t,
                    (lambda qc=qc, t=kt_next: t[:, qc*512:(qc+1)*512]))
                fills.extend(mms); fills.append(evac)
            if j == NJ - 1 and qc > 0:
                for scl in range(4):
                    fills.extend(o_proj_sc((qc - 1) * 4 + scl))

            pvh0 = ps_pv.tile([65, 512], F32, tag="pv")
            pvh1 = ps_pv.tile([65, 512], F32, tag="pv")
            nf = len(fills)
            pending = []

            def emit_pv(pg, pk):
                nc.tensor.matmul(
                    pvh0, vt[:, pk, j, 0, :],
                    pg[:, 0, :], start=(pk == 0), stop=(pk == NKC - 1))
                nc.tensor.matmul(
                    pvh1, vt[:, pk, j, 1, :],
                    pg[:, 1, :], start=(pk == 0), stop=(pk == NKC - 1))

            for k in range(NKC):
                sgrp = ps_sc.tile([128, 2, 512], F32, tag="sc")
                nc.tensor.matmul(
                    sgrp[:, 0, :], kt[0:64, k*128:(k+1)*128],
                    qt[0:64, :], start=True, stop=True)
                nc.tensor.matmul(
                    sgrp[:, 1, :], kt[64:128, k*128:(k+1)*128],
                    qt[64:128, :], start=True, stop=True)
                pgrp = ppool.tile([128, 2, 512], BF16, tag="p")
                # Split the exp load: ACT does 3/4, DVE does head1 on odd
                # k-chunks via the Schraudolph bit-trick (bf16 bits written
                # through an int16 view). DVE reads PSUM at 1 elem/cycle,
                # so only a quarter of the stream fits in its slack.
                if k % 2 == 1:
                    nc.scalar.activation(pgrp[:, 0, :], sgrp[:, 0, :],
                                         EXP, scale=0.125)
                    nc.vector.tensor_scalar(
                        pgrp[:, 1, :].bitcast(I16), sgrp[:, 1, :],
                        SCH_A, SCH_B,
                        mybir.AluOpType.mult, mybir.AluOpType.add)
                else:
                    nc.scalar.activation(pgrp[:, :, :], sgrp[:, :, :],
                                         EXP, scale=0.125)
                # two-iteration skew: PV for chunk k-2 behind scores/exp k
                pending.append((pgrp, k))
                if len(pending) > 2:
                    emit_pv(*pending.pop(0))
                # drain fill work: ceil-spread across the 16 k slots
                lo = (nf * k) // NKC
                hi = (nf * (k + 1)) // NKC
                for f in fills[lo:hi]:
                    f()
            for pg, pk in pending:
                emit_pv(pg, pk)
            den0 = dpool.tile([1, 512], F32R, tag="den0")
            nc.vector.tensor_copy(den0, pvh0[64:65, :])
            den1 = dpool.tile([1, 512], F32R, tag="den1")
            nc.vector.tensor_copy(den1, pvh1[64:65, :])
            drep0 = ps_proj.tile([64, 512], F32, tag="proj")
            nc.tensor.matmul(drep0, ones64[0:1, :], den0,
                             start=True, stop=True)
            drep1 = ps_proj.tile([64, 512], F32, tag="proj")
            nc.tensor.matmul(drep1, ones64[0:1, :], den1,
                             start=True, stop=True)
            recip0 = dpool.tile([64, 512], F32, tag="recip")
            nc.vector.reciprocal_approx_fast(out=recip0, in_=drep0)
            recip1 = dpool.tile([64, 512], F32, tag="recip1")
            nc.vector.reciprocal_approx_fast(out=recip1, in_=drep1)
            nc.vector.tensor_mul(
                oct_[0:64, j, qc*512:(qc+1)*512], pvh0[0:64, :], recip0)
            nc.vector.tensor_scalar_add(
                oct_[0:64, j, qc*512:(qc+1)*512],
                oct_[0:64, j, qc*512:(qc+1)*512], bv_t[0:64, j:j+1])
            tmp1 = dpool.tile([64, 512], BF16, tag="tmp1")
            nc.vector.tensor_mul(tmp1, pvh1[0:64, :], recip1)
            nc.vector.tensor_scalar_add(tmp1, tmp1, bvh1_t[0:64, j:j+1])
            nc.sync.dma_start(out=oct_[64:128, j, qc*512:(qc+1)*512], in_=tmp1)

    # last q-chunk's output projection (tail)
    for scl in range(4):
        for f in o_proj_sc(12 + scl):
            f()


def _get_nc():
    if "nc" not in _CACHED:
        _CACHED["nc"] = _build()
    return _CACHED["nc"]


def build_in_maps(inputs):
    """Per-core input maps (host-side shard + bf16 conversion)."""
    x = np.asarray(inputs["x"], dtype=np.float32)
    Wq = np.asarray(inputs["Wq"], dtype=np.float32)
    Wk = np.asarray(inputs["Wk"], dtype=np.float32)
    Wv = np.asarray(inputs["Wv"], dtype=np.float32)
    Wo = np.asarray(inputs["Wo"], dtype=np.float32)
    bq = np.asarray(inputs["bq"], dtype=np.float32)
    bk = np.asarray(inputs["bk"], dtype=np.float32)
    bv = np.asarray(inputs["bv"], dtype=np.float32)
    in_maps = []
    for c in range(8):
        b, hh = c // 2, c % 2
        hsel = slice(hh * DLOC, (hh + 1) * DLOC)
        in_maps.append({
            "xT": np.ascontiguousarray(x[b].T).astype(NPBF16),
            "wqT": np.ascontiguousarray(Wq[hsel, :].T).astype(NPBF16),
            "wkT": np.ascontiguousarray(Wk[hsel, :].T).astype(NPBF16),
            "wvT": np.ascontiguousarray(Wv[hsel, :].T).astype(NPBF16),
            "woT": np.ascontiguousarray(Wo[:, hsel].T).astype(NPBF16),
            "bq": bq[hsel].reshape(DLOC, 1),
            "bk": bk[hsel].reshape(DLOC, 1),
            "bv": bv[hsel].reshape(DLOC, 1),
            "ones": np.ones((128, 64), dtype=np.float32),
        })
    return in_maps


def kernel(x, Wq, bq, Wk, bk, Wv, bv, Wo, bo):
    in_maps = build_in_maps({"x": x, "Wq": Wq, "bq": bq, "Wk": Wk, "bk": bk,
                             "Wv": Wv, "bv": bv, "Wo": Wo})
    nc = _get_nc()
    res = run_bass_kernel_spmd(nc, in_maps, list(range(8))).results
    out = np.empty((B, S, E), dtype=np.float32)
    bo = np.asarray(bo, dtype=np.float32)
    for b in range(B):
        out[b] = res[2 * b]["y"] + res[2 * b + 1]["y"] + bo
    return out


# revision 40
# speedup vs baseline: 1.7393x; 1.7393x over previous
"""Trainium2 Bass kernel for 16-head MHA (B=4, S=2048, E=1024, fp32 io).

Sharding: 8 cores = (batch b, head-half hh) grid. Core c handles batch
c // 2 and heads [hh*8, hh*8+8) (d-slice of 512 channels). Each core
computes a partial y_c = attn_out_slice @ Wo_slice.T of the full (S, E)
output; the host sums core pairs and adds bo.

Layout choices (bf16 data, fp32 PSUM accumulation):
  - All inputs (xT, weights) are converted to bf16 host-side: halves
    HBM traffic, SBUF footprint and DVE evac cost; PE matmul throughput
    is dtype-independent at these tile shapes (1 cycle/row).
  - One strided DMA per weight/bias tensor (each dma_start costs
    >=625ns regardless of size); constants queued behind wv/xT so the
    first V-proj matmul isn't stuck behind them on the DMA queue.
  - V projection runs e-outer in two 8-bank PSUM passes (borrowing the
    then-idle k-loop pools' banks) so matmuls start as soon as the
    first xT e-chunk lands instead of waiting for the full xT DMA.
  - Scores (K=64 per head) are emitted as a row-tiled pair: head0 on
    PE rows 0-63, head1 on rows 64-127 (tile_position auto-derived
    from base partitions) -> the two matmuls run concurrently.
  - k-loop is software-pipelined with a two-iteration skew (pgrp pool
    6 deep): PV for chunk k-2 is emitted after scores+exp of chunk k,
    so the PE never waits on the current exp (ACT) and both engines
    stream. (Measured optimum: skew 1/3 and pool 3/8 are all slower.)
  - Softmax denominator rides the PV matmul as a 65th output row
    (ones column appended to V); normalization via ones-matmul
    replicate + reciprocal + multiply on DVE. (GpSimd partition
    broadcast and DMA broadcast variants measured far slower.)
  - Q/K projections and the output projection are spread as fill work
    across the k-loop slots.
"""
import numpy as np
import ml_dtypes

import concourse.bass as bass
import concourse.mybir as mybir
import concourse.tile as tile
from concourse import bacc
from concourse.bass_utils import run_bass_kernel_spmd

B, S, E = 4, 2048, 1024
DLOC = 512          # head-dim channels per core (8 heads)
NJ = DLOC // 128    # 4 j-chunks (head pairs)
NE = E // 128       # 8 e-chunks
NSC = S // 128      # 16 s-chunks
NQC = S // 512      # 4 q-chunks
NKC = S // 128      # 16 k-chunks
F32 = mybir.dt.float32
F32R = mybir.dt.float32r
BF16 = mybir.dt.bfloat16
EXP = mybir.ActivationFunctionType.Exp
NPBF16 = ml_dtypes.bfloat16

_CACHED = {}


def _build(loop_k=None):
    nc = bacc.Bacc()
    xT = nc.declare_dram_parameter("xT", [E, S], BF16, isOutput=False)
    wqT = nc.declare_dram_parameter("wqT", [E, DLOC], BF16, isOutput=False)
    wkT = nc.declare_dram_parameter("wkT", [E, DLOC], BF16, isOutput=False)
    wvT = nc.declare_dram_parameter("wvT", [E, DLOC], BF16, isOutput=False)
    woT = nc.declare_dram_parameter("woT", [DLOC, E], BF16, isOutput=False)
    bq = nc.declare_dram_parameter("bq", [DLOC, 1], F32, isOutput=False)
    bk = nc.declare_dram_parameter("bk", [DLOC, 1], F32, isOutput=False)
    bv = nc.declare_dram_parameter("bv", [DLOC, 1], F32, isOutput=False)
    ones = nc.declare_dram_parameter("ones", [128, 64], F32R, isOutput=False)
    y = nc.declare_dram_parameter("y", [S, E], F32, isOutput=True)

    with tile.TileContext(nc) as tc:
        with (
            tc.tile_pool(name="big", bufs=1) as big,
            tc.tile_pool(name="wpool", bufs=1) as wpool,
            tc.tile_pool(name="cons", bufs=1) as cons,
            tc.tile_pool(name="qpool", bufs=1) as qpool,
            tc.tile_pool(name="opool", bufs=2) as opool,
            tc.tile_pool(name="ppool", bufs=6) as ppool,
            tc.tile_pool(name="dpool", bufs=1) as dpool,
            tc.tile_pool(name="ypool", bufs=1) as ypool,
            tc.tile_pool(name="ps_proj", bufs=2, space="PSUM") as ps_proj,
            tc.tile_pool(name="ps_sc", bufs=2, space="PSUM") as ps_sc,
            tc.tile_pool(name="ps_pv", bufs=2, space="PSUM") as ps_pv,
        ):
            # ---- constants (single strided DMA per tensor) ----
            ones64 = cons.tile([128, 64], F32R)
            bq_t = cons.tile([128, NJ], F32)
            bk_t = cons.tile([128, NJ], F32)
            bv_t = cons.tile([128, NJ], F32)
            bvh1_t = cons.tile([64, NJ], F32)

            def load_consts():
                nc.sync.dma_start(out=ones64, in_=ones[:, :])
                nc.sync.dma_start(
                    out=bq_t, in_=bq.rearrange("(j p) o -> p (j o)", p=128))
                nc.sync.dma_start(
                    out=bk_t, in_=bk.rearrange("(j p) o -> p (j o)", p=128))
                nc.sync.dma_start(
                    out=bv_t, in_=bv.rearrange("(j p) o -> p (j o)", p=128))
                nc.sync.dma_start(
                    out=bvh1_t,
                    in_=bv.rearrange("(j h p) o -> h p (j o)", j=NJ, h=2)[1])

            # ---- optional on-device repeat loop (timing only) ----
            import contextlib
            loop_cm = tc.For_i(0, loop_k) if loop_k else contextlib.nullcontext()
            with loop_cm:
                _body(nc, tc, locals())

    nc.compile()
    return nc


def _body(nc, tc, env):
    xT, wqT, wkT, wvT, woT = env["xT"], env["wqT"], env["wkT"], env["wvT"], env["woT"]
    y = env["y"]
    big, wpool, qpool, opool = env["big"], env["wpool"], env["qpool"], env["opool"]
    ppool, dpool, ypool = env["ppool"], env["dpool"], env["ypool"]
    ps_proj, ps_sc, ps_pv = env["ps_proj"], env["ps_sc"], env["ps_pv"]
    ones64 = env["ones64"]
    bq_t, bk_t, bv_t, bvh1_t = (env["bq_t"], env["bk_t"], env["bv_t"],
                                env["bvh1_t"])

    # ---- load wv, then xT chunk-by-chunk (V proj overlaps the DMA) ----
    wv_t = wpool.tile([128, NE, DLOC], BF16, tag="wv")
    nc.sync.dma_start(out=wv_t, in_=wvT.rearrange("(e p) d -> p e d", p=128))
    xt_list = []
    for e in range(NE):
        xe = big.tile([128, S], BF16, tag=f"xt{e}", bufs=2)
        nc.sync.dma_start(out=xe, in_=xT[e*128:(e+1)*128, :])
        xt_list.append(xe)
    wk_t = wpool.tile([128, NE, DLOC], BF16, tag="wk")
    nc.sync.dma_start(out=wk_t, in_=wkT.rearrange("(e p) d -> p e d", p=128))
    wq_t = wpool.tile([128, NE, DLOC], BF16, tag="wq")
    nc.sync.dma_start(out=wq_t, in_=wqT.rearrange("(e p) d -> p e d", p=128))
    wo_t = wpool.tile([128, NJ, E], BF16, tag="wo")
    nc.sync.dma_start(out=wo_t, in_=woT.rearrange("(j p) d -> p j d", p=128))
    env["load_consts"]()  # queued behind wv/xT so they don't delay V-proj

    # ---- V projection, e-outer in two 8-bank passes ----
    # vt[:, sc, j, h, 0:64] = V columns; vt[:, sc, j, h, 64] = 1.0 so the
    # PV matmul's 65th output row accumulates the softmax denominator.
    vt = big.tile([128, NSC, NJ, 2, 65], BF16)
    nc.vector.memset(vt[:, :, :, :, 64:65], 1.0)
    # The k-loop PSUM pools are idle during the prologue, so borrow their
    # banks for eight concurrent e-outer accumulation groups per pass.
    for half in range(2):
        t1 = ps_sc.tile([128, 2, 512], F32, tag="sc")
        t2 = ps_sc.tile([128, 2, 512], F32, tag="sc")
        t3 = ps_pv.tile([128, 512], F32, tag="pv")
        t4 = ps_pv.tile([128, 512], F32, tag="pv")
        t5 = ps_proj.tile([128, 512], F32, tag="proj")
        t6 = ps_proj.tile([128, 512], F32, tag="proj")
        groups = [t1[:, 0, :], t1[:, 1, :], t2[:, 0, :], t2[:, 1, :],
                  t3, t4, t5, t6]
        for e in range(NE):
            for scl in range(8):
                sc = half * 8 + scl
                nc.tensor.matmul(
                    groups[scl], xt_list[e][:, sc*128:(sc+1)*128],
                    wv_t[:, e, :], start=(e == 0), stop=(e == NE - 1))
        for scl in range(8):
            sc = half * 8 + scl
            nc.vector.tensor_copy(
                vt[:, sc, :, :, 0:64],
                groups[scl].rearrange("p (j h c) -> p j h c", j=NJ, h=2))

    oct_ = big.tile([128, NJ, S], BF16)

    # ---- main loop: j (head pairs) outer, q-chunks inner ----
    # Projections are software-pipelined into the attention k-loop ("fill"
    # slots) so the statically-scheduled PE stream never starves ACT.
    def k_proj_group(j, qc, w_t, b_t, dest_fn, ps_fn=None):
        cell = {}
        def get_pk():
            if "pk" not in cell:
                if ps_fn is not None:
                    cell["pk"] = ps_fn()
                else:
                    pk_lazy = ps_proj.tile([128, 512], F32, tag="proj")
                    cell["pk"] = pk_lazy
            return cell["pk"]
        mms = [lambda e=e: nc.tensor.matmul(
                   get_pk(), w_t[:, e, j*128:(j+1)*128],
                   xt_list[e][:, qc*512:(qc+1)*512],
                   start=(e == 0), stop=(e == NE - 1)) for e in range(NE)]
        def evac():
            nc.vector.tensor_scalar_add(dest_fn(), get_pk(), b_t[:, j:j+1])
        return mms, evac

    def o_proj_sc(sc):
        cell = {}
        def get_ysb():
            if "ysb" not in cell:
                ysb_lazy = ypool.tile([128, E], F32, tag="y", bufs=2)
                cell["ysb"] = ysb_lazy
            return cell["ysb"]
        def get_py(eh):
            key = f"py{eh}"
            if key not in cell:
                py_lazy = ps_proj.tile([128, 512], F32, tag="proj")
                cell[key] = py_lazy
            return cell[key]
        steps = []
        for eh in range(2):
            for jj in range(NJ):
                steps.append(lambda jj=jj, eh=eh: nc.tensor.matmul(
                    get_py(eh), oct_[:, jj, sc*128:(sc+1)*128],
                    wo_t[:, jj, eh*512:(eh+1)*512],
                    start=(jj == 0), stop=(jj == NJ - 1)))
            steps.append(lambda eh=eh: nc.vector.tensor_copy(
                get_ysb()[:, eh*512:(eh+1)*512], get_py(eh)))
        steps.append(lambda: nc.sync.dma_start(
            out=y[sc*128:(sc+1)*128, :], in_=get_ysb()))
        return steps

    # K-projection for j=0 and Q-projection for (0, 0) run up front.
    # Borrow the idle scores-psum slots so all five groups pipeline
    # without waiting on the two ps_proj slots.
    ka = ps_sc.tile([128, 2, 512], F32, tag="sc")
    kb = ps_sc.tile([128, 2, 512], F32, tag="sc")
    kt_next = qpool.tile([128, S], BF16, tag="kt", bufs=2)
    for qc in range(NQC):
        mms, evac = k_proj_group(
            0, qc, wk_t, bk_t,
            (lambda qc=qc, t=kt_next: t[:, qc*512:(qc+1)*512]),
            ps_fn=(lambda qc=qc: (ka, kb)[qc // 2][:, qc % 2, :]))
        for m in mms:
            m()
        evac()
    qt_next = qpool.tile([128, 512], BF16, tag="qt", bufs=2)
    mms, evac = k_proj_group(
        0, 0, wq_t, bq_t, (lambda t=qt_next: t[:, :]))
    for m in mms:
        m()
    evac()

    for j in range(NJ):
        kt = kt_next
        if j < NJ - 1:
            kt_next = qpool.tile([128, S], BF16, tag="kt", bufs=2)
        for qc in range(NQC):
            qt = qt_next
            # fill work emitted one step per k iteration
            fills = []
            if qc < NQC - 1:
                qt_next = qpool.tile([128, 512], BF16, tag="qt", bufs=2)
                mms, evac = k_proj_group(
                    j, qc + 1, wq_t, bq_t, (lambda t=qt_next: t[:, :]))
                fills.extend(mms); fills.append(evac)
            elif j < NJ - 1:
                qt_next = qpool.tile([128, 512], BF16, tag="qt", bufs=2)
                mms, evac = k_proj_group(
                    j + 1, 0, wq_t, bq_t, (lambda t=qt_next: t[:, :]))
                fills.extend(mms); fills.append(evac)
            if j < NJ - 1:
                mms, evac = k_proj_group(
                    j + 1, qc, wk_t, bk_t,
                    (lambda qc=qc, t=kt_next: t[:, qc*512:(qc+1)*512]))
                fills.extend(mms); fills.append(evac)
            if j == NJ - 1 and qc > 0:
                for scl in range(4):
                    fills.extend(o_proj_sc((qc - 1) * 4 + scl))

            pvh0 = ps_pv.tile([65, 512], F32, tag="pv")
            pvh1 = ps_pv.tile([65, 512], F32, tag="pv")
            nf = len(fills)
            pending = []

            def emit_pv(pg, pk):
                nc.tensor.matmul(
                    pvh0, vt[:, pk, j, 0, :],
                    pg[:, 0, :], start=(pk == 0), stop=(pk == NKC - 1))
                nc.tensor.matmul(
                    pvh1, vt[:, pk, j, 1, :],
                    pg[:, 1, :], start=(pk == 0), stop=(pk == NKC - 1))

            for k in range(NKC):
                sgrp = ps_sc.tile([128, 2, 512], F32, tag="sc")
                nc.tensor.matmul(
                    sgrp[:, 0, :], kt[0:64, k*128:(k+1)*128],
                    qt[0:64, :], start=True, stop=True)
                nc.tensor.matmul(
                    sgrp[:, 1, :], kt[64:128, k*128:(k+1)*128],
                    qt[64:128, :], start=True, stop=True)
                pgrp = ppool.tile([128, 2, 512], BF16, tag="p")
                nc.scalar.activation(pgrp[:, :, :], sgrp[:, :, :],
                                     EXP, scale=0.125)
                # two-iteration skew: PV for chunk k-2 behind scores/exp k
                pending.append((pgrp, k))
                if len(pending) > 2:
                    emit_pv(*pending.pop(0))
                # drain fill work: ceil-spread across the 16 k slots
                lo = (nf * k) // NKC
                hi = (nf * (k + 1)) // NKC
                for f in fills[lo:hi]:
                    f()
            for pg, pk in pending:
                emit_pv(pg, pk)
            den0 = dpool.tile([1, 512], F32R, tag="den0")
            nc.vector.tensor_copy(den0, pvh0[64:65, :])
            den1 = dpool.tile([1, 512], F32R, tag="den1")
            nc.vector.tensor_copy(den1, pvh1[64:65, :])
            drep0 = ps_proj.tile([64, 512], F32, tag="proj")
            nc.tensor.matmul(drep0, ones64[0:1, :], den0,
                             start=True, stop=True)
            drep1 = ps_proj.tile([64, 512], F32, tag="proj")
            nc.tensor.matmul(drep1, ones64[0:1, :], den1,
                             start=True, stop=True)
            recip0 = dpool.tile([64, 512], F32, tag="recip")
            nc.vector.reciprocal_approx_fast(out=recip0, in_=drep0)
            recip1 = dpool.tile([64, 512], F32, tag="recip1")
            nc.vector.reciprocal_approx_fast(out=recip1, in_=drep1)
            nc.vector.tensor_mul(
                oct_[0:64, j, qc*512:(qc+1)*512], pvh0[0:64, :], recip0)
            nc.vector.tensor_scalar_add(
                oct_[0:64, j, qc*512:(qc+1)*512],
                oct_[0:64, j, qc*512:(qc+1)*512], bv_t[0:64, j:j+1])
            tmp1 = dpool.tile([64, 512], BF16, tag="tmp1")
            nc.vector.tensor_mul(tmp1, pvh1[0:64, :], recip1)
            nc.vector.tensor_scalar_add(tmp1, tmp1, bvh1_t[0:64, j:j+1])
            nc.sync.dma_start(out=oct_[64:128, j, qc*512:(qc+1)*512], in_=tmp1)

    # last q-chunk's output projection (tail)
    for scl in range(4):
        for f in o_proj_sc(12 + scl):
            f()


def _get_nc():
    if "nc" not in _CACHED:
        _CACHED["nc"] = _build()
    return _CACHED["nc"]


def build_in_maps(inputs):
    """Per-core input maps (host-side shard + bf16 conversion)."""
    x = np.asarray(inputs["x"], dtype=np.float32)
    Wq = np.asarray(inputs["Wq"], dtype=np.float32)
    Wk = np.asarray(inputs["Wk"], dtype=np.float32)
    Wv = np.asarray(inputs["Wv"], dtype=np.float32)
    Wo = np.asarray(inputs["Wo"], dtype=np.float32)
    bq = np.asarray(inputs["bq"], dtype=np.float32)
    bk = np.asarray(inputs["bk"], dtype=np.float32)
    bv = np.asarray(inputs["bv"], dtype=np.float32)
    in_maps = []
    for c in range(8):
        b, hh = c // 2, c % 2
        hsel = slice(hh * DLOC, (hh + 1) * DLOC)
        in_maps.append({
            "xT": np.ascontiguousarray(x[b].T).astype(NPBF16),
            "wqT": np.ascontiguousarray(Wq[hsel, :].T).astype(NPBF16),
            "wkT": np.ascontiguousarray(Wk[hsel, :].T).astype(NPBF16),
            "wvT": np.ascontiguousarray(Wv[hsel, :].T).astype(NPBF16),
            "woT": np.ascontiguousarray(Wo[:, hsel].T).astype(NPBF16),
            "bq": bq[hsel].reshape(DLOC, 1),
            "bk": bk[hsel].reshape(DLOC, 1),
            "bv": bv[hsel].reshape(DLOC, 1),
            "ones": np.ones((128, 64), dtype=np.float32),
        })
    return in_maps


def kernel(x, Wq, bq, Wk, bk, Wv, bv, Wo, bo):
    in_maps = build_in_maps({"x": x, "Wq": Wq, "bq": bq, "Wk": Wk, "bk": bk,
                             "Wv": Wv, "bv": bv, "Wo": Wo})
    nc = _get_nc()
    res = run_bass_kernel_spmd(nc, in_maps, list(range(8))).results
    out = np.empty((B, S, E), dtype=np.float32)
    bo = np.asarray(bo, dtype=np.float32)
    for b in range(B):
        out[b] = res[2 * b]["y"] + res[2 * b + 1]["y"] + bo
    return out
